# revision 31
# baseline (speedup 1.0000x reference)
"""Trainium2 kernel for per-subject linear heads (moe_routing).

Computes out[i] = x[i] @ W[subject_ids[i]] + b[subject_ids[i]] for
B=256, D=2048, S=8 subjects, OUT=1000.

Sharding: expert-parallel — core s owns subject s. Each core reads only
its own (2048, 1000) weight slice from HBM, so the total weight traffic
across the chip is W read exactly once (vs 8x for batch-data-parallel
with a replicated table). Samples are grouped by subject on the host,
padded to a fixed capacity C, and fed to an SPMD Bass/Tile kernel;
outputs are scattered back to the original order.

The kernel is HBM-bound: the host casts x/W/b to fp16, halving the
stream to ~4.4 MB/core (~12.5 us at the ~358 GB/s per-core HBM limit).
fp16 keeps 10 mantissa bits, so the dot-product rel err stays ~3e-4 —
well inside the 2e-2 gate. PSUM accumulates in fp32 and y is fp32.

Kernel-side notes:
- The bias is folded into the matmul accumulation as a rank-1 update
  (ones row carried as an extra k-slot of x, times the [1, OUT] bias).
- This walrus build rejects any instruction with more than one sync
  wait, so the kernel is structured so no instruction ever needs two:
  a tiny absorber matmul (reads only x) carries the x-DMA wait, so the
  bias matmuls wait only on the bias DMA and each chunk's first matmul
  waits only on that chunk's completion-sem lane.
- fp16 matmuls stream 1 cycle/column (vs 4 for fp32) and the two
  500-wide n-tiles run concurrently on disjoint PE column groups
  (tile_position col 0 / col 64, C <= 64 rows each), so the PE keeps
  pace with the DMA stream even at the cold 1.2 GHz clock — no HAM
  warm-up spins needed.
- W is pre-permuted on the host so each chunk DMA reads one contiguous
  4 KB run per partition.
"""

import numpy as np

import concourse.bass as bass
import concourse.mybir as mybir
import concourse.tile as tile
from concourse.bass_utils import run_bass_kernel_spmd
from concourse.vector_clock import ScopedClock, VectorClock

B = 256
D = 2048
S = 8
OUT = 1000
P = 128
KO = D // P          # 16 k-tiles of 128
NT = 500             # psum n-tile (<= 512 fp32 / bank), 2 tiles cover OUT
# W DMA chunks as (first k-tile, span, ring): 512 KB fp16 mains plus
# 256 KB tail minis on BOTH rings, so each ring's final completion-sem
# lag and matmul burst cover only 256 KB. Ring 0 = SP (also carries x,
# starts ~0.7 us before ACT), ring 1 = ACT; byte totals per ring are
# balanced (SP 2.26 MB incl x, ACT 2.05 MB) and the interleave matches
# the expected arrival order so the k-ordered matmul stream never waits
# on an out-of-order chunk.
CHUNKS = [
    (0, 2, 1), (2, 2, 0), (4, 2, 1), (6, 2, 0), (8, 2, 1), (10, 2, 0),
    (12, 1, 0), (13, 1, 1), (14, 1, 1),
]
N_CHUNKS = len(CHUNKS)
# The final k-tile (ko15) is split into per-n-tile 128 KB half-chunks:
# when the n0 half lands, n0's closing matmul, PSUM->SBUF copy, and y0
# SWDGE generation all run while the n1 half is still in flight — the
# serial post-stream tail then covers only 128 KB.
LAST_KO = KO - 1
LAST_RINGS = (0, 1)  # ko15-n0 on SP, ko15-n1 on ACT

TRACE = False        # set by test harness to collect an NTFF profile
LAST_RESULTS = None  # BassKernelResults of the most recent run

_nc_cache = {}


class _FastExitTileContext(tile.TileContext):
    """TileContext with a no-op exit: no drains, no clears, no barriers.

    The stock exit (drain every semaphore + two all-engine butterfly
    barriers + GpSimd semaphore clears) exists so a re-execution of the
    NEFF starts from zeroed semaphores. Both halves of that are already
    guaranteed elsewhere in this build: the Bass preamble dma_resets and
    sem_clears the whole kernel semaphore range at NEFF START, and the
    walrus codegen epilogue re-zeros every semaphore (3..255, split
    across engines) at NEFF END. So the Tile exit can simply fall
    through to the walrus epilogue. That matters for latency: the
    epilogue opens with an all-engine barrier, so its ~6 us semaphore
    wall starts at the LAST engine's last instruction — with drains that
    is SP after the y-write completion sems (~2.5 us after the y DMA
    trigger); without them it is the y trigger itself. The y data lands
    ~1 us into the ~7 us epilogue, comfortably before the NEFF
    completes and outputs are read back.
    """

    def _drain_and_barrier(self, tick_clock, wait_clock):
        nc = self.nc
        assert self.sems is not None
        popped = nc._tile_sem_poison_stack.pop()
        assert popped is self._sem_poison
        nc._state.prepend_free_semaphores(
            [h.num for h in self.sems.allocated().values()]
        )


def _build(C):
    """Per-core program: y[C, OUT] = xT.T @ w + bias.

    xT   : [P, KO+1, C] fp16     xT[p, ko, c] = x_subject[c, ko*P + p]
                                 for ko < KO; last slot all-ones (bias).
    w    : [N_CHUNKS, P, CH*OUT] fp16 host-permuted weights;
                                 w[ch, p, j*OUT+n] = W[(ch*CH+j)*P + p, n].
    bias : [1, OUT] fp16         the subject's bias row.
    """
    cdt = mybir.dt.float16
    nc = bass.Bass(enable_partition_id=False)
    xT = nc.dram_tensor("xT", [P, KO + 1, C], cdt, kind="ExternalInput")
    w_drams = [
        nc.dram_tensor(f"w{ci}", [P, span * OUT], cdt, kind="ExternalInput")
        for ci, (a, span, ring) in enumerate(CHUNKS)
    ]
    wlast_drams = [
        nc.dram_tensor(f"wlast{n}", [P, NT], cdt, kind="ExternalInput")
        for n in range(2)
    ]
    bias = nc.dram_tensor("bias", [1, OUT], cdt, kind="ExternalInput")
    y = nc.dram_tensor("y", [C, OUT], mybir.dt.float32, kind="ExternalOutput")

    m_tiles = [(m0, min(P, C - m0)) for m0 in range(0, C, P)]
    # For mc <= 64 the two n-tiles share one PSUM bank on disjoint
    # column halves of the PE array and run concurrently.
    col_tiled = all(mc <= 64 for _, mc in m_tiles)

    with _FastExitTileContext(nc) as tc:
        with (
            tc.tile_pool(name="wpool", bufs=N_CHUNKS + 2) as wpool,
            tc.tile_pool(name="xpool", bufs=1) as xpool,
            tc.tile_pool(name="bpool", bufs=1) as bpool,
            tc.tile_pool(name="opool", bufs=4) as opool,
            tc.tile_pool(name="psum", bufs=1, space="PSUM") as psum_pool,
        ):
            # x first on SP, then the W chunks on their assigned rings
            # (see CHUNKS). The 2 KB bias rides the SWDGE (gpsimd)
            # queue: a tiny DMA at the head of an HWDGE ring stalls that
            # ring ~2.5 us while its completion receipt round-trips, so
            # keep it off the weight stream entirely. HWDGE
            # completion-sem lanes round-robin over 8 in issue order;
            # the lane-sharing late chunks' first matmuls wait
            # "lane >= 32" — still a single wait each.
            x_tile = xpool.tile([P, KO + 1, C], cdt)
            nc.sync.dma_start(x_tile[:], xT[:])
            b_tile = bpool.tile([1, OUT], cdt)
            nc.gpsimd.dma_start(b_tile[:], bias[:])

            rings = [nc.sync, nc.scalar]
            w_tiles = []
            for ci, (a, span, ring) in enumerate(CHUNKS):
                wt = wpool.tile([P, span * OUT], cdt)
                rings[ring].dma_start(wt[:], w_drams[ci][:])
                w_tiles.append(wt)
            wlast_tiles = []
            for n in range(2):
                wt = wpool.tile([P, NT], cdt)
                rings[LAST_RINGS[n]].dma_start(wt[:], wlast_drams[n][:])
                wlast_tiles.append(wt)

            psums = {}
            tilepos = {}
            joints = []
            for mi, (m0, mc) in enumerate(m_tiles):
                if col_tiled:
                    joint = psum_pool.tile(
                        [P, NT], mybir.dt.float32, name=f"psum_{mi}"
                    )
                    joints.append(joint)
                    psums[(mi, 0)] = joint[0:mc]
                    psums[(mi, 1)] = joint[64 : 64 + mc]
                    tilepos[(mi, 0)] = (0, 0)
                    tilepos[(mi, 1)] = (0, 64)
                else:
                    for n in range(2):
                        psums[(mi, n)] = psum_pool.tile(
                            [mc, NT], mybir.dt.float32, name=f"psum_{mi}_{n}"
                        )
                        tilepos[(mi, n)] = None

            # Absorber: the only PE instruction that waits on the x DMA.
            # Later matmuls reading x_tile inherit the wait via the Tile
            # vector clock, so each needs only its own bias/chunk wait.
            absorb = psum_pool.tile([1, C], mybir.dt.float32, name="absorb")
            nc.tensor.matmul(
                absorb[:, :],
                x_tile[0:1, KO, 0:1],
                x_tile[0:1, KO, :],
                start=True,
                stop=True,
            )
            # Open each accumulation group with the rank-1 bias update:
            # ones[1, mc].T @ bias[1, NT].
            for mi, (m0, mc) in enumerate(m_tiles):
                for n in range(2):
                    nc.tensor.matmul(
                        psums[(mi, n)][:, :],
                        x_tile[0:1, KO, m0 : m0 + mc],
                        b_tile[0:1, n * NT : (n + 1) * NT],
                        start=True,
                        stop=False,
                        tile_position=tilepos[(mi, n)],
                    )
            # k-contiguous loop over ko 0..14: each W chunk is consumed
            # for every (m, n) output tile as soon as it lands, then is
            # dead.
            for ci, (a, span, ring) in enumerate(CHUNKS):
                wt = w_tiles[ci]
                for j in range(span):
                    ko = a + j
                    base = j * OUT
                    for mi, (m0, mc) in enumerate(m_tiles):
                        lhsT = x_tile[:, ko, m0 : m0 + mc]
                        for n in range(2):
                            nc.tensor.matmul(
                                psums[(mi, n)][:, :],
                                lhsT,
                                wt[:, base + n * NT : base + (n + 1) * NT],
                                start=False,
                                stop=False,
                                tile_position=tilepos[(mi, n)],
                            )
            # ko15 arrives as per-n 128 KB half-chunks, so the closing
            # matmuls start as soon as each half lands and the serial
            # post-stream tail covers only the n1 half. (A fully per-n
            # drain pipeline is blocked by Tile's tile-granularity
            # hazard on the shared PSUM bank: interleaving copy-n0
            # before matmul-n1 adds a false WAR wait and walrus rejects
            # the second sync wait.) Drain: in the col-split layout both
            # n-tiles live in ONE PSUM bank (partitions 0..mc-1 and
            # 64..64+mc-1), so a single [64+mc, NT] DVE copy moves both
            # at once; two SWDGE y DMAs then scatter the halves (each
            # waits only the DVE sem and stays off the 8 HWDGE
            # completion-sem lanes).
            for n in range(2):
                for mi, (m0, mc) in enumerate(m_tiles):
                    nc.tensor.matmul(
                        psums[(mi, n)][:, :],
                        x_tile[:, LAST_KO, m0 : m0 + mc],
                        wlast_tiles[n][:, :],
                        start=False,
                        stop=True,
                        tile_position=tilepos[(mi, n)],
                    )
            for mi, (m0, mc) in enumerate(m_tiles):
                if col_tiled:
                    ot = opool.tile([64 + mc, NT], mybir.dt.float32)
                    nc.vector.tensor_copy(ot[:], joints[mi][0 : 64 + mc])
                    nc.gpsimd.dma_start(y[m0 : m0 + mc, 0:NT], ot[0:mc])
                    nc.gpsimd.dma_start(
                        y[m0 : m0 + mc, NT : 2 * NT], ot[64 : 64 + mc]
                    )
                else:
                    ot = opool.tile([mc, 2 * NT], mybir.dt.float32)
                    for n in range(2):
                        nc.vector.tensor_copy(
                            ot[:, n * NT : (n + 1) * NT], psums[(mi, n)][:]
                        )
                    nc.gpsimd.dma_start(y[m0 : m0 + mc, :], ot[:])
    return nc


def _capacity(max_count):
    c = 48
    while c < max_count:
        c += 16
    return c


def kernel(x, subject_ids, W, b):
    global LAST_RESULTS
    x = np.ascontiguousarray(np.asarray(x, dtype=np.float32))
    sid = np.asarray(subject_ids).astype(np.int64)
    W = np.ascontiguousarray(np.asarray(W, dtype=np.float32))
    b = np.ascontiguousarray(np.asarray(b, dtype=np.float32))

    groups = [np.nonzero(sid == s)[0] for s in range(S)]
    C = _capacity(max((len(g) for g in groups), default=1))

    key = (C, tuple(CHUNKS))
    if key not in _nc_cache:
        _nc_cache[key] = _build(C)
    nc = _nc_cache[key]

    # Per chunk (a, span): [p, j*OUT + n] = W[s, (a + j)*P + p, n] — one
    # contiguous span*2KB run per partition per chunk DMA.
    W16 = W.astype(np.float16).reshape(S, KO, P, OUT)
    W_chunks = [
        np.ascontiguousarray(
            W16[:, a : a + span].transpose(0, 2, 1, 3).reshape(S, P, span * OUT)
        )
        for (a, span, ring) in CHUNKS
    ]
    W_last = [
        np.ascontiguousarray(W16[:, LAST_KO, :, n * NT : (n + 1) * NT])
        for n in range(2)
    ]
    b16 = b.astype(np.float16)

    in_maps = []
    for s in range(S):
        idx = groups[s]
        xs = np.zeros((C, D), dtype=np.float32)
        xs[: len(idx)] = x[idx]
        # [p, ko, c] = xs[c, ko*P + p]; extra all-ones k-slot for bias
        xT = np.empty((P, KO + 1, C), dtype=np.float16)
        xT[:, :KO, :] = xs.T.reshape(KO, P, C).transpose(1, 0, 2)
        xT[:, KO, :] = 1.0
        im = {"xT": xT, "bias": b16[s : s + 1]}
        for ci in range(N_CHUNKS):
            im[f"w{ci}"] = W_chunks[ci][s]
        for n in range(2):
            im[f"wlast{n}"] = W_last[n][s]
        in_maps.append(im)

    LAST_RESULTS = run_bass_kernel_spmd(
        nc, in_maps, core_ids=list(range(S)), trace=TRACE
    )

    out = np.zeros((B, OUT), dtype=np.float32)
    for s in range(S):
        idx = groups[s]
        out[idx] = LAST_RESULTS.results[s]["y"][: len(idx)]
    return out


# revision 38
# speedup vs baseline: 1.0340x; 1.0340x over previous
"""Trainium2 kernel for per-subject linear heads (moe_routing).

Computes out[i] = x[i] @ W[subject_ids[i]] + b[subject_ids[i]] for
B=256, D=2048, S=8 subjects, OUT=1000.

Sharding: expert-parallel — core s owns subject s. Each core reads only
its own (2048, 1000) weight slice from HBM, so the total weight traffic
across the chip is W read exactly once (vs 8x for batch-data-parallel
with a replicated table). Samples are grouped by subject on the host,
padded to a fixed capacity C, and fed to an SPMD Bass/Tile kernel;
outputs are scattered back to the original order.

The kernel is HBM-bound: the host casts x/W/b to fp16, halving the
stream to ~4.4 MB/core (~12.5 us at the ~358 GB/s per-core HBM limit).
fp16 keeps 10 mantissa bits, so the dot-product rel err stays ~3e-4 —
well inside the 2e-2 gate. PSUM accumulates in fp32 and y is fp32.

Kernel-side notes:
- The bias is folded into the matmul accumulation as a rank-1 update
  (ones row carried as an extra k-slot of x, times the [1, OUT] bias).
- This walrus build rejects any instruction with more than one sync
  wait, so the kernel is structured so no instruction ever needs two:
  a tiny absorber matmul (reads only x) carries the x-DMA wait, so the
  bias matmuls wait only on the bias DMA and each chunk's first matmul
  waits only on that chunk's completion-sem lane.
- fp16 matmuls stream 1 cycle/column (vs 4 for fp32) and the two
  500-wide n-tiles run concurrently on disjoint PE column groups
  (tile_position col 0 / col 64, C <= 64 rows each), so the PE keeps
  pace with the DMA stream even at the cold 1.2 GHz clock — no HAM
  warm-up spins needed.
- W is pre-permuted on the host so each chunk DMA reads one contiguous
  4 KB run per partition.
"""

import numpy as np

import concourse.bass as bass
import concourse.mybir as mybir
import concourse.tile as tile
from concourse.bass_utils import run_bass_kernel_spmd
from concourse.vector_clock import ScopedClock, VectorClock

B = 256
D = 2048
S = 8
OUT = 1000
P = 128
KO = D // P          # 16 k-tiles of 128
NT = 500             # psum n-tile (<= 512 fp32 / bank), 2 tiles cover OUT
# W DMA chunks as (first k-tile, span, ring): 512 KB fp16 mains plus
# 256 KB tail minis on BOTH rings, so each ring's final completion-sem
# lag and matmul burst cover only 256 KB. Ring 0 = SP (also carries x,
# starts ~0.7 us before ACT), ring 1 = ACT; byte totals per ring are
# balanced (SP 2.26 MB incl x, ACT 2.05 MB) and the interleave matches
# the expected arrival order so the k-ordered matmul stream never waits
# on an out-of-order chunk.
# Exactly x + 7 W chunks = 8 HWDGE DMAs -> zero completion-sem lane
# reuse. A 9th+ HWDGE DMA must wait for the receipt (~2.4 us after last
# byte) of the DMA 8 issues earlier before it can even ISSUE, and if
# the ring's queued bytes don't cover that stall the stream bubbles
# (measured: 12 DMAs with small tails stretched the stream 2 us).
CHUNKS = [
    (0, 2, 1), (2, 3, 0), (5, 3, 1), (8, 3, 0), (11, 2, 1), (13, 2, 0),
    (15, 1, 1),
]
N_CHUNKS = len(CHUNKS)
LAST_KO = KO - 1

TRACE = False        # set by test harness to collect an NTFF profile
LAST_RESULTS = None  # BassKernelResults of the most recent run

_nc_cache = {}


class _FastExitTileContext(tile.TileContext):
    """TileContext with a no-op exit: no drains, no clears, no barriers.

    The stock exit (drain every semaphore + two all-engine butterfly
    barriers + GpSimd semaphore clears) exists so a re-execution of the
    NEFF starts from zeroed semaphores. Both halves of that are already
    guaranteed elsewhere in this build: the Bass preamble dma_resets and
    sem_clears the whole kernel semaphore range at NEFF START, and the
    walrus codegen epilogue re-zeros every semaphore (3..255, split
    across engines) at NEFF END. So the Tile exit can simply fall
    through to the walrus epilogue. That matters for latency: the
    epilogue opens with an all-engine barrier, so its ~6 us semaphore
    wall starts at the LAST engine's last instruction — with drains that
    is SP after the y-write completion sems (~2.5 us after the y DMA
    trigger); without them it is the y trigger itself. The y data lands
    ~1 us into the ~7 us epilogue, comfortably before the NEFF
    completes and outputs are read back.
    """

    def _drain_and_barrier(self, tick_clock, wait_clock):
        nc = self.nc
        assert self.sems is not None
        popped = nc._tile_sem_poison_stack.pop()
        assert popped is self._sem_poison
        nc._state.prepend_free_semaphores(
            [h.num for h in self.sems.allocated().values()]
        )


def _build(C):
    """Per-core program: y[C, OUT] = xT.T @ w + bias.

    xT   : [P, KO+1, C] fp16     xT[p, ko, c] = x_subject[c, ko*P + p]
                                 for ko < KO; last slot all-ones (bias).
    w    : [N_CHUNKS, P, CH*OUT] fp16 host-permuted weights;
                                 w[ch, p, j*OUT+n] = W[(ch*CH+j)*P + p, n].
    bias : [1, OUT] fp16         the subject's bias row.
    """
    cdt = mybir.dt.float16
    nc = bass.Bass(enable_partition_id=False)
    xT = nc.dram_tensor("xT", [P, KO + 1, C], cdt, kind="ExternalInput")
    w_drams = [
        nc.dram_tensor(f"w{ci}", [P, span * OUT], cdt, kind="ExternalInput")
        for ci, (a, span, ring) in enumerate(CHUNKS)
    ]
    bias = nc.dram_tensor("bias", [1, OUT], cdt, kind="ExternalInput")
    y = nc.dram_tensor("y", [C, OUT], mybir.dt.float32, kind="ExternalOutput")

    m_tiles = [(m0, min(P, C - m0)) for m0 in range(0, C, P)]
    # For mc <= 64 the two n-tiles share one PSUM bank on disjoint
    # column halves of the PE array and run concurrently.
    col_tiled = all(mc <= 64 for _, mc in m_tiles)

    with _FastExitTileContext(nc) as tc:
        with (
            tc.tile_pool(name="wpool", bufs=N_CHUNKS) as wpool,
            tc.tile_pool(name="xpool", bufs=1) as xpool,
            tc.tile_pool(name="bpool", bufs=1) as bpool,
            tc.tile_pool(name="opool", bufs=4) as opool,
            tc.tile_pool(name="psum", bufs=1, space="PSUM") as psum_pool,
        ):
            # x first on SP, then the W chunks on their assigned rings
            # (see CHUNKS). The 2 KB bias rides the SWDGE (gpsimd)
            # queue: a tiny DMA at the head of an HWDGE ring stalls that
            # ring ~2.5 us while its completion receipt round-trips, so
            # keep it off the weight stream entirely. HWDGE
            # completion-sem lanes round-robin over 8 in issue order;
            # the lane-sharing late chunks' first matmuls wait
            # "lane >= 32" — still a single wait each.
            x_tile = xpool.tile([P, KO + 1, C], cdt)
            nc.sync.dma_start(x_tile[:], xT[:])
            b_tile = bpool.tile([1, OUT], cdt)
            nc.gpsimd.dma_start(b_tile[:], bias[:])

            rings = [nc.sync, nc.scalar]
            w_tiles = []
            for ci, (a, span, ring) in enumerate(CHUNKS):
                wt = wpool.tile([P, span * OUT], cdt)
                rings[ring].dma_start(wt[:], w_drams[ci][:])
                w_tiles.append(wt)

            psums = {}
            tilepos = {}
            joints = []
            for mi, (m0, mc) in enumerate(m_tiles):
                if col_tiled:
                    joint = psum_pool.tile(
                        [P, NT], mybir.dt.float32, name=f"psum_{mi}"
                    )
                    joints.append(joint)
                    psums[(mi, 0)] = joint[0:mc]
                    psums[(mi, 1)] = joint[64 : 64 + mc]
                    tilepos[(mi, 0)] = (0, 0)
                    tilepos[(mi, 1)] = (0, 64)
                else:
                    for n in range(2):
                        psums[(mi, n)] = psum_pool.tile(
                            [mc, NT], mybir.dt.float32, name=f"psum_{mi}_{n}"
                        )
                        tilepos[(mi, n)] = None

            # Absorber: the only PE instruction that waits on the x DMA.
            # Later matmuls reading x_tile inherit the wait via the Tile
            # vector clock, so each needs only its own bias/chunk wait.
            absorb = psum_pool.tile([1, C], mybir.dt.float32, name="absorb")
            nc.tensor.matmul(
                absorb[:, :],
                x_tile[0:1, KO, 0:1],
                x_tile[0:1, KO, :],
                start=True,
                stop=True,
            )
            # Open each accumulation group with the rank-1 bias update:
            # ones[1, mc].T @ bias[1, NT].
            for mi, (m0, mc) in enumerate(m_tiles):
                for n in range(2):
                    nc.tensor.matmul(
                        psums[(mi, n)][:, :],
                        x_tile[0:1, KO, m0 : m0 + mc],
                        b_tile[0:1, n * NT : (n + 1) * NT],
                        start=True,
                        stop=False,
                        tile_position=tilepos[(mi, n)],
                    )
            # k-contiguous loop: each W chunk is consumed for every
            # (m, n) output tile as soon as it lands, then is dead.
            for ci, (a, span, ring) in enumerate(CHUNKS):
                wt = w_tiles[ci]
                for j in range(span):
                    ko = a + j
                    base = j * OUT
                    for mi, (m0, mc) in enumerate(m_tiles):
                        lhsT = x_tile[:, ko, m0 : m0 + mc]
                        for n in range(2):
                            nc.tensor.matmul(
                                psums[(mi, n)][:, :],
                                lhsT,
                                wt[:, base + n * NT : base + (n + 1) * NT],
                                start=False,
                                stop=(ko == KO - 1),
                                tile_position=tilepos[(mi, n)],
                            )
            # Drain: in the col-split layout both n-tiles live in ONE
            # PSUM bank (partitions 0..mc-1 and 64..64+mc-1), so a
            # single [64+mc, NT] DVE copy moves both at once; two SWDGE
            # y DMAs then scatter the halves (each waits only the DVE
            # sem and stays off the 8 HWDGE completion-sem lanes).
            for mi, (m0, mc) in enumerate(m_tiles):
                if col_tiled:
                    ot = opool.tile([64 + mc, NT], mybir.dt.float32)
                    nc.vector.tensor_copy(ot[:], joints[mi][0 : 64 + mc])
                    nc.gpsimd.dma_start(y[m0 : m0 + mc, 0:NT], ot[0:mc])
                    nc.gpsimd.dma_start(
                        y[m0 : m0 + mc, NT : 2 * NT], ot[64 : 64 + mc]
                    )
                else:
                    ot = opool.tile([mc, 2 * NT], mybir.dt.float32)
                    for n in range(2):
                        nc.vector.tensor_copy(
                            ot[:, n * NT : (n + 1) * NT], psums[(mi, n)][:]
                        )
                    nc.gpsimd.dma_start(y[m0 : m0 + mc, :], ot[:])
    return nc


def _capacity(max_count):
    c = 48
    while c < max_count:
        c += 16
    return c


def kernel(x, subject_ids, W, b):
    global LAST_RESULTS
    x = np.ascontiguousarray(np.asarray(x, dtype=np.float32))
    sid = np.asarray(subject_ids).astype(np.int64)
    W = np.ascontiguousarray(np.asarray(W, dtype=np.float32))
    b = np.ascontiguousarray(np.asarray(b, dtype=np.float32))

    groups = [np.nonzero(sid == s)[0] for s in range(S)]
    C = _capacity(max((len(g) for g in groups), default=1))

    key = (C, tuple(CHUNKS))
    if key not in _nc_cache:
        _nc_cache[key] = _build(C)
    nc = _nc_cache[key]

    # Per chunk (a, span): [p, j*OUT + n] = W[s, (a + j)*P + p, n] — one
    # contiguous span*2KB run per partition per chunk DMA.
    W16 = W.astype(np.float16).reshape(S, KO, P, OUT)
    W_chunks = [
        np.ascontiguousarray(
            W16[:, a : a + span].transpose(0, 2, 1, 3).reshape(S, P, span * OUT)
        )
        for (a, span, ring) in CHUNKS
    ]
    b16 = b.astype(np.float16)

    in_maps = []
    for s in range(S):
        idx = groups[s]
        xs = np.zeros((C, D), dtype=np.float32)
        xs[: len(idx)] = x[idx]
        # [p, ko, c] = xs[c, ko*P + p]; extra all-ones k-slot for bias
        xT = np.empty((P, KO + 1, C), dtype=np.float16)
        xT[:, :KO, :] = xs.T.reshape(KO, P, C).transpose(1, 0, 2)
        xT[:, KO, :] = 1.0
        im = {"xT": xT, "bias": b16[s : s + 1]}
        for ci in range(N_CHUNKS):
            im[f"w{ci}"] = W_chunks[ci][s]
        in_maps.append(im)

    LAST_RESULTS = run_bass_kernel_spmd(
        nc, in_maps, core_ids=list(range(S)), trace=TRACE
    )

    out = np.zeros((B, OUT), dtype=np.float32)
    for s in range(S):
        idx = groups[s]
        out[idx] = LAST_RESULTS.results[s]["y"][: len(idx)]
    return out


# revision 39
# speedup vs baseline: 1.0938x; 1.0578x over previous
"""Trainium2 kernel for per-subject linear heads (moe_routing).

Computes out[i] = x[i] @ W[subject_ids[i]] + b[subject_ids[i]] for
B=256, D=2048, S=8 subjects, OUT=1000.

Sharding: expert-parallel — core s owns subject s. Each core reads only
its own (2048, 1000) weight slice from HBM, so the total weight traffic
across the chip is W read exactly once (vs 8x for batch-data-parallel
with a replicated table). Samples are grouped by subject on the host,
padded to a fixed capacity C, and fed to an SPMD Bass/Tile kernel;
outputs are scattered back to the original order.

The kernel is HBM-bound: the host casts x/W/b to fp16, halving the
stream to ~4.4 MB/core (~12.5 us at the ~358 GB/s per-core HBM limit).
fp16 keeps 10 mantissa bits, so the dot-product rel err stays ~3e-4 —
well inside the 2e-2 gate. PSUM accumulates in fp32 and y is fp32.

Kernel-side notes:
- The bias is folded into the matmul accumulation as a rank-1 update
  (ones row carried as an extra k-slot of x, times the [1, OUT] bias).
- This walrus build rejects any instruction with more than one sync
  wait, so the kernel is structured so no instruction ever needs two:
  a tiny absorber matmul (reads only x) carries the x-DMA wait, so the
  bias matmuls wait only on the bias DMA and each chunk's first matmul
  waits only on that chunk's completion-sem lane.
- fp16 matmuls stream 1 cycle/column (vs 4 for fp32) and the two
  500-wide n-tiles run concurrently on disjoint PE column groups
  (tile_position col 0 / col 64, C <= 64 rows each), so the PE keeps
  pace with the DMA stream even at the cold 1.2 GHz clock — no HAM
  warm-up spins needed.
- W is pre-permuted on the host so each chunk DMA reads one contiguous
  4 KB run per partition.
"""

import numpy as np

import concourse.bass as bass
import concourse.mybir as mybir
import concourse.tile as tile
from concourse.bass_utils import run_bass_kernel_spmd
from concourse.vector_clock import ScopedClock, VectorClock

B = 256
D = 2048
S = 8
OUT = 1000
P = 128
KO = D // P          # 16 k-tiles of 128
NT = 500             # psum n-tile (<= 512 fp32 / bank), 2 tiles cover OUT
# W DMA chunks as (first k-tile, span, ring): 512 KB fp16 mains plus
# 256 KB tail minis on BOTH rings, so each ring's final completion-sem
# lag and matmul burst cover only 256 KB. Ring 0 = SP (also carries x,
# starts ~0.7 us before ACT), ring 1 = ACT; byte totals per ring are
# balanced (SP 2.26 MB incl x, ACT 2.05 MB) and the interleave matches
# the expected arrival order so the k-ordered matmul stream never waits
# on an out-of-order chunk.
# Chunk geometry notes (all HW-measured on this problem):
# - >11 HWDGE DMAs hurts: a DMA on a reused completion-sem lane can't
#   ISSUE until the receipt (~2.4 us after last byte) of the DMA 8
#   issues earlier; with small tail chunks the ring runs dry during
#   the stall and the stream bubbles (+2 us).
# - The two rings must END STAGGERED (~0.7 us here via SP's extra x
#   bytes): each SDMA engine stalls on the write-receipt of a DMA's
#   final sem descriptor, which is hidden only while the OTHER queue
#   still has data. Equal ring ends trickle the last ~130 KB over
#   ~2.5 us (+1.4 us).
# - 256 KB minis at both rings' tails keep the final completion-sem
#   lag and closing matmul burst small.
CHUNKS = [
    (0, 2, 1), (2, 2, 0), (4, 2, 1), (6, 2, 0), (8, 2, 1), (10, 2, 0),
    (12, 1, 0), (13, 1, 1), (14, 1, 1), (15, 1, 0),
]
N_CHUNKS = len(CHUNKS)
LAST_KO = KO - 1

TRACE = False        # set by test harness to collect an NTFF profile
LAST_RESULTS = None  # BassKernelResults of the most recent run

_nc_cache = {}


class _FastExitTileContext(tile.TileContext):
    """TileContext with a no-op exit: no drains, no clears, no barriers.

    The stock exit (drain every semaphore + two all-engine butterfly
    barriers + GpSimd semaphore clears) exists so a re-execution of the
    NEFF starts from zeroed semaphores. Both halves of that are already
    guaranteed elsewhere in this build: the Bass preamble dma_resets and
    sem_clears the whole kernel semaphore range at NEFF START, and the
    walrus codegen epilogue re-zeros every semaphore (3..255, split
    across engines) at NEFF END. So the Tile exit can simply fall
    through to the walrus epilogue. That matters for latency: the
    epilogue opens with an all-engine barrier, so its ~6 us semaphore
    wall starts at the LAST engine's last instruction — with drains that
    is SP after the y-write completion sems (~2.5 us after the y DMA
    trigger); without them it is the y trigger itself. The y data lands
    ~1 us into the ~7 us epilogue, comfortably before the NEFF
    completes and outputs are read back.
    """

    def _drain_and_barrier(self, tick_clock, wait_clock):
        nc = self.nc
        assert self.sems is not None
        popped = nc._tile_sem_poison_stack.pop()
        assert popped is self._sem_poison
        nc._state.prepend_free_semaphores(
            [h.num for h in self.sems.allocated().values()]
        )


def _build(C):
    """Per-core program: y[C, OUT] = xT.T @ w + bias.

    xT   : [P, KO+1, C] fp16     xT[p, ko, c] = x_subject[c, ko*P + p]
                                 for ko < KO; last slot all-ones (bias).
    w    : [N_CHUNKS, P, CH*OUT] fp16 host-permuted weights;
                                 w[ch, p, j*OUT+n] = W[(ch*CH+j)*P + p, n].
    bias : [1, OUT] fp16         the subject's bias row.
    """
    cdt = mybir.dt.float16
    nc = bass.Bass(enable_partition_id=False)
    xT = nc.dram_tensor("xT", [P, KO + 1, C], cdt, kind="ExternalInput")
    w_drams = [
        nc.dram_tensor(f"w{ci}", [P, span * OUT], cdt, kind="ExternalInput")
        for ci, (a, span, ring) in enumerate(CHUNKS)
    ]
    bias = nc.dram_tensor("bias", [1, OUT], cdt, kind="ExternalInput")
    y = nc.dram_tensor("y", [C, OUT], mybir.dt.float32, kind="ExternalOutput")

    m_tiles = [(m0, min(P, C - m0)) for m0 in range(0, C, P)]
    # For mc <= 64 the two n-tiles share one PSUM bank on disjoint
    # column halves of the PE array and run concurrently.
    col_tiled = all(mc <= 64 for _, mc in m_tiles)

    with _FastExitTileContext(nc) as tc:
        with (
            tc.tile_pool(name="wpool", bufs=N_CHUNKS) as wpool,
            tc.tile_pool(name="xpool", bufs=1) as xpool,
            tc.tile_pool(name="bpool", bufs=1) as bpool,
            tc.tile_pool(name="opool", bufs=4) as opool,
            tc.tile_pool(name="psum", bufs=1, space="PSUM") as psum_pool,
        ):
            # x first on SP, then the W chunks on their assigned rings
            # (see CHUNKS). The 2 KB bias rides the SWDGE (gpsimd)
            # queue: a tiny DMA at the head of an HWDGE ring stalls that
            # ring ~2.5 us while its completion receipt round-trips, so
            # keep it off the weight stream entirely. HWDGE
            # completion-sem lanes round-robin over 8 in issue order;
            # the lane-sharing late chunks' first matmuls wait
            # "lane >= 32" — still a single wait each.
            x_tile = xpool.tile([P, KO + 1, C], cdt)
            nc.sync.dma_start(x_tile[:], xT[:])
            b_tile = bpool.tile([1, OUT], cdt)
            nc.gpsimd.dma_start(b_tile[:], bias[:])

            rings = [nc.sync, nc.scalar]
            w_tiles = []
            for ci, (a, span, ring) in enumerate(CHUNKS):
                wt = wpool.tile([P, span * OUT], cdt)
                rings[ring].dma_start(wt[:], w_drams[ci][:])
                w_tiles.append(wt)

            psums = {}
            tilepos = {}
            joints = []
            for mi, (m0, mc) in enumerate(m_tiles):
                if col_tiled:
                    joint = psum_pool.tile(
                        [P, NT], mybir.dt.float32, name=f"psum_{mi}"
                    )
                    joints.append(joint)
                    psums[(mi, 0)] = joint[0:mc]
                    psums[(mi, 1)] = joint[64 : 64 + mc]
                    tilepos[(mi, 0)] = (0, 0)
                    tilepos[(mi, 1)] = (0, 64)
                else:
                    for n in range(2):
                        psums[(mi, n)] = psum_pool.tile(
                            [mc, NT], mybir.dt.float32, name=f"psum_{mi}_{n}"
                        )
                        tilepos[(mi, n)] = None

            # Absorber: the only PE instruction that waits on the x DMA.
            # Later matmuls reading x_tile inherit the wait via the Tile
            # vector clock, so each needs only its own bias/chunk wait.
            absorb = psum_pool.tile([1, C], mybir.dt.float32, name="absorb")
            nc.tensor.matmul(
                absorb[:, :],
                x_tile[0:1, KO, 0:1],
                x_tile[0:1, KO, :],
                start=True,
                stop=True,
            )
            # Open each accumulation group with the rank-1 bias update:
            # ones[1, mc].T @ bias[1, NT].
            for mi, (m0, mc) in enumerate(m_tiles):
                for n in range(2):
                    nc.tensor.matmul(
                        psums[(mi, n)][:, :],
                        x_tile[0:1, KO, m0 : m0 + mc],
                        b_tile[0:1, n * NT : (n + 1) * NT],
                        start=True,
                        stop=False,
                        tile_position=tilepos[(mi, n)],
                    )
            # k-contiguous loop: each W chunk is consumed for every
            # (m, n) output tile as soon as it lands, then is dead.
            for ci, (a, span, ring) in enumerate(CHUNKS):
                wt = w_tiles[ci]
                for j in range(span):
                    ko = a + j
                    base = j * OUT
                    for mi, (m0, mc) in enumerate(m_tiles):
                        lhsT = x_tile[:, ko, m0 : m0 + mc]
                        for n in range(2):
                            nc.tensor.matmul(
                                psums[(mi, n)][:, :],
                                lhsT,
                                wt[:, base + n * NT : base + (n + 1) * NT],
                                start=False,
                                stop=(ko == KO - 1),
                                tile_position=tilepos[(mi, n)],
                            )
            # Drain: in the col-split layout both n-tiles live in ONE
            # PSUM bank (partitions 0..mc-1 and 64..64+mc-1), so a
            # single [64+mc, NT] DVE copy moves both at once; two SWDGE
            # y DMAs then scatter the halves (each waits only the DVE
            # sem and stays off the 8 HWDGE completion-sem lanes).
            for mi, (m0, mc) in enumerate(m_tiles):
                if col_tiled:
                    ot = opool.tile([64 + mc, NT], mybir.dt.float32)
                    nc.vector.tensor_copy(ot[:], joints[mi][0 : 64 + mc])
                    nc.gpsimd.dma_start(y[m0 : m0 + mc, 0:NT], ot[0:mc])
                    nc.gpsimd.dma_start(
                        y[m0 : m0 + mc, NT : 2 * NT], ot[64 : 64 + mc]
                    )
                else:
                    ot = opool.tile([mc, 2 * NT], mybir.dt.float32)
                    for n in range(2):
                        nc.vector.tensor_copy(
                            ot[:, n * NT : (n + 1) * NT], psums[(mi, n)][:]
                        )
                    nc.gpsimd.dma_start(y[m0 : m0 + mc, :], ot[:])
    return nc


def _capacity(max_count):
    c = 48
    while c < max_count:
        c += 16
    return c


def kernel(x, subject_ids, W, b):
    global LAST_RESULTS
    x = np.ascontiguousarray(np.asarray(x, dtype=np.float32))
    sid = np.asarray(subject_ids).astype(np.int64)
    W = np.ascontiguousarray(np.asarray(W, dtype=np.float32))
    b = np.ascontiguousarray(np.asarray(b, dtype=np.float32))

    groups = [np.nonzero(sid == s)[0] for s in range(S)]
    C = _capacity(max((len(g) for g in groups), default=1))

    key = (C, tuple(CHUNKS))
    if key not in _nc_cache:
        _nc_cache[key] = _build(C)
    nc = _nc_cache[key]

    # Per chunk (a, span): [p, j*OUT + n] = W[s, (a + j)*P + p, n] — one
    # contiguous span*2KB run per partition per chunk DMA.
    W16 = W.astype(np.float16).reshape(S, KO, P, OUT)
    W_chunks = [
        np.ascontiguousarray(
            W16[:, a : a + span].transpose(0, 2, 1, 3).reshape(S, P, span * OUT)
        )
        for (a, span, ring) in CHUNKS
    ]
    b16 = b.astype(np.float16)

    in_maps = []
    for s in range(S):
        idx = groups[s]
        xs = np.zeros((C, D), dtype=np.float32)
        xs[: len(idx)] = x[idx]
        # [p, ko, c] = xs[c, ko*P + p]; extra all-ones k-slot for bias
        xT = np.empty((P, KO + 1, C), dtype=np.float16)
        xT[:, :KO, :] = xs.T.reshape(KO, P, C).transpose(1, 0, 2)
        xT[:, KO, :] = 1.0
        im = {"xT": xT, "bias": b16[s : s + 1]}
        for ci in range(N_CHUNKS):
            im[f"w{ci}"] = W_chunks[ci][s]
        in_maps.append(im)

    LAST_RESULTS = run_bass_kernel_spmd(
        nc, in_maps, core_ids=list(range(S)), trace=TRACE
    )

    out = np.zeros((B, OUT), dtype=np.float32)
    for s in range(S):
        idx = groups[s]
        out[idx] = LAST_RESULTS.results[s]["y"][: len(idx)]
    return out
